# revision 1
# baseline (speedup 1.0000x reference)
"""FK velocity loss kernel for Trainium2 (8 NeuronCores, SPMD data parallel).

Math notes (derived from the reference loss):
  * Each 72-float sample holds 8 joints x 3x3 "m".  The 4x4 joint matrix is
    M = [[cross(c0,c1), c0, c1, c2], [0,0,0,1]] with ck = column k of m.
  * FK translation of a 4-joint chain:  z = R0 (R1 (R2 t3 + t2) + t1) + t0
    with Rj = [xj, c0j, c1j], tj = c2j, xj = cross(c0j, c1j).
    Right-association avoids all 3x3 @ 3x3 products and never needs the
    cross product of the depth-3 joints.
  * vel_loss == pos_loss exactly:
    (out_fk - prev) - (gt_fk - prev) = out_fk - gt_fk.  Therefore
    gt_prev_pose does not influence either loss value and is never read.

Engine split per (tile, tensor):
  * DVE: only the multiplies (cross products' 2 muls/sub stay on DVE; the
    chain's 3 term-products per step).
  * TensorE: all chain additions, as bit-exact identity matmuls accumulating
    into PSUM (fp32 x 1.0 through the PE array is exact; verified on HW).
  * ScalarE: copies final z from PSUM to SBUF and issues ring-B DMAs.
  * GpSimd is useless here: its SBUF port is shared with the DVE's second
    read port, so concurrent gpsimd tensor ops serialize against DVE.

Device computes the per-sample FK translations z for output_pose/gt_pose and
ships them to the host; the host does the (order-independent) mean of squared
differences in float64.
"""

import numpy as np

import concourse.bass as bass
import concourse.bacc as bacc
import concourse.tile as tile
from concourse import mybir
from concourse.bass_utils import run_bass_kernel_spmd

B = 262144
N_CORES = 8
PER_CORE = B // N_CORES        # 32768 samples per core
P = 128                        # SBUF partitions
COLS = PER_CORE // P           # 256 samples per partition per core
F32 = mybir.dt.float32

# Tile plan: S values; sum must equal COLS.  Each tile covers 128*S samples.
DEFAULT_PLAN = (64, 64, 64, 64)


def _lead(ap, step, count):
    """Insert a free dim (step in elements) right after the partition dim."""
    return bass.AP(
        tensor=ap.tensor,
        offset=ap.offset,
        ap=[ap.ap[0], [step, count]] + list(ap.ap[1:]),
    )


def build_nc(cols=COLS, plan=DEFAULT_PLAN, loop=None, no_dma=False,
             dma_only=False, use_pe="dve_t", m_bufs=4, split_loads="col", fat=False,
             t_bufs=3, x_bufs=3, s_bufs=3, p_bufs=6):
    assert sum(plan) == cols
    per_core = cols * P

    nc = bacc.Bacc()
    src_out = nc.declare_dram_parameter("output_pose", [per_core, 72], F32, isOutput=False)
    src_gt = nc.declare_dram_parameter("gt_pose", [per_core, 72], F32, isOutput=False)
    ident_in = nc.declare_dram_parameter("ident", [P, P], F32, isOutput=False)
    z_out = nc.declare_dram_parameter("z_out", [P, 6 * cols], F32, isOutput=True)
    z_gt = nc.declare_dram_parameter("z_gt", [P, 6 * cols], F32, isOutput=True)

    with tile.TileContext(nc) as tc:
        import contextlib

        loop_ctx = tc.For_i(0, loop, 1) if loop else contextlib.nullcontext()
        with (
            loop_ctx,
            tc.tile_pool(name="singles", bufs=1) as singles,
            tc.tile_pool(name="m_pool", bufs=m_bufs) as mpool,
            tc.tile_pool(name="x_pool", bufs=x_bufs) as xpool,
            tc.tile_pool(name="small", bufs=s_bufs) as spool,
            tc.tile_pool(name="term", bufs=t_bufs) as tpool,
            tc.tile_pool(name="psum", bufs=p_bufs, space="PSUM") as ppool,
            tc.tile_pool(name="z_io", bufs=8) as z_io,
        ):
            if use_pe:
                # SWDGE ring: keeps the ident load off the head of the
                # sync HWDGE FIFO, where it would delay the first m load
                ident = singles.tile([P, P], F32)
                nc.gpsimd.dma_start(out=ident[:], in_=ident_in[:])
            eng = nc.vector
            m_shared = None
            if no_dma:
                # compute-only benchmarking: all tiles read one memset tile
                shape = [P, 2, max(plan), 2, 4, 9] if fat else [P, max(plan), 2, 4, 9]
                m_shared = singles.tile(shape, F32)
                nc.vector.memset(m_shared[:], 1.0)
            col_base = 0
            if fat:
                # both pose tensors batched into every instruction
                for S in plan:
                    row0 = col_base * P
                    if no_dma:
                        m = m_shared
                    else:
                        m = mpool.tile([P, 2, S, 2, 4, 9], F32, tag="m")
                    if not no_dma:
                        for a, (src, ring) in enumerate(
                            ((src_out, nc.sync), (src_gt, nc.scalar))
                        ):
                            ring.dma_start(
                                out=m[:, a].rearrange("p s c d k -> p (s c d k)"),
                                in_=src[row0 : row0 + P * S, :].rearrange(
                                    "(p s) f -> p (s f)", p=P
                                ),
                            )
                    if dma_only:
                        col_base += S
                        continue

                    # cross products, both tensors per instruction
                    x = xpool.tile([P, 3, 2, S, 2, 3], F32, tag="x")
                    t12 = spool.tile([P, 2, S, 2, 3], F32, tag="t12", bufs=1)
                    for r in range(3):
                        r1, r2 = (r + 1) % 3, (r + 2) % 3
                        eng.tensor_mul(
                            x[:, r],
                            m[:, :, :, :, 0:3, 3 * r1],
                            m[:, :, :, :, 0:3, 3 * r2 + 1],
                        )
                        eng.tensor_mul(
                            t12[:],
                            m[:, :, :, :, 0:3, 3 * r2],
                            m[:, :, :, :, 0:3, 3 * r1 + 1],
                        )
                        eng.tensor_sub(x[:, r], x[:, r], t12[:])

                    va = spool.tile([P, 3, 2, S, 2], F32, tag="va", bufs=1)
                    vb = spool.tile([P, 3, 2, S, 2], F32, tag="vb", bufs=1)
                    tm = spool.tile([P, 3, 2, S, 2], F32, tag="tm", bufs=1)
                    z = z_io.tile([P, 3, 2, S, 2], F32, tag="z", bufs=4)

                    def fcolr(d, c):
                        return _lead(m[:, :, :, :, d, c], 3, 3)

                    def fstep(d, vin, vout):
                        eng.tensor_mul(
                            vout[:], x[:, :, :, :, :, d], _lead(vin[0], 0, 3)
                        )
                        eng.tensor_mul(tm[:], fcolr(d, 0), _lead(vin[1], 0, 3))
                        eng.tensor_add(vout[:], vout[:], tm[:])
                        eng.tensor_mul(tm[:], fcolr(d, 1), _lead(vin[2], 0, 3))
                        eng.tensor_add(vout[:], vout[:], tm[:])
                        eng.tensor_add(vout[:], vout[:], fcolr(d, 2))

                    t3 = [m[:, :, :, :, 3, 3 * k + 2] for k in range(3)]
                    fstep(2, t3, va)
                    fstep(1, [va[:, k] for k in range(3)], vb)
                    fstep(0, [vb[:, k] for k in range(3)], z)

                    for a, zdst in enumerate((z_out, z_gt)):
                        nc.gpsimd.dma_start(
                            out=zdst[:, 6 * col_base : 6 * (col_base + S)].rearrange(
                                "p (r s c) -> p r s c", r=3, s=S
                            ),
                            in_=z[:, :, a],
                        )
                    col_base += S
                plan = ()  # skip the per-tensor path below
            for S in plan:
                row0 = col_base * P
                for ti, (src, zdst) in enumerate(((src_out, z_out), (src_gt, z_gt))):
                    dma_eng = nc.sync if ti == 0 else nc.scalar
                    # m layout per partition: S samples x (chain 2, depth 4, k 9)
                    if no_dma:
                        m = m_shared
                    else:
                        m = mpool.tile([P, S, 2, 4, 9], F32, tag="m")
                    if not no_dma:
                        flat = m[:].rearrange("p s c d k -> p (s c d k)")
                        srcv = src[row0 : row0 + P * S, :].rearrange(
                            "(p s) f -> p (s f)", p=P
                        )
                        if split_loads == "col":
                            # split each load across both HWDGE rings (SP + ACT)
                            h = (S // 2) * 72
                            nc.sync.dma_start(out=flat[:, :h], in_=srcv[:, :h])
                            nc.scalar.dma_start(out=flat[:, h:], in_=srcv[:, h:])
                        elif split_loads == "col3":
                            # thirds across SP + ACT HWDGE and gpsimd SWDGE
                            h1 = (S // 3) * 72
                            h2 = (2 * S // 3) * 72
                            nc.sync.dma_start(out=flat[:, :h1], in_=srcv[:, :h1])
                            nc.scalar.dma_start(out=flat[:, h1:h2], in_=srcv[:, h1:h2])
                            nc.gpsimd.dma_start(out=flat[:, h2:], in_=srcv[:, h2:])
                        elif split_loads == "part":
                            # partition split: ring A drives SDMA engines 0-7,
                            # ring B engines 8-15 (engine k owns partitions 8k..)
                            nc.sync.dma_start(out=flat[0:64, :], in_=srcv[0:64, :])
                            nc.scalar.dma_start(out=flat[64:128, :], in_=srcv[64:128, :])
                        else:
                            dma_eng.dma_start(out=flat[:], in_=srcv[:])
                    if dma_only:
                        continue

                    # cross products for depths 0..2 of both chains (DVE)
                    # x[r] = m[3*r1]*m[3*r2+1] - m[3*r2]*m[3*r1+1], rk=(r+k)%3
                    x = xpool.tile([P, 3, S, 2, 3], F32, tag="x")
                    tmp6 = spool.tile([P, S, 2, 3], F32, tag="t6")
                    for r in range(3):
                        r1, r2 = (r + 1) % 3, (r + 2) % 3
                        eng.tensor_mul(
                            x[:, r], m[:, :, :, 0:3, 3 * r1], m[:, :, :, 0:3, 3 * r2 + 1]
                        )
                        eng.tensor_mul(
                            tmp6[:], m[:, :, :, 0:3, 3 * r2], m[:, :, :, 0:3, 3 * r1 + 1]
                        )
                        eng.tensor_sub(x[:, r], x[:, r], tmp6[:])

                    def colr(d, c):
                        # [P, 3(r), S, 2]: element r of column c of joint depth d
                        return _lead(m[:, :, :, d, c], 3, 3)

                    # chain: v <- R_d v + t_d for d = 2, 1, 0 (v init = t3)
                    unit_idx = (col_base // S if S else 0) * 2 + ti
                    mode = use_pe
                    if use_pe == "hybrid":
                        # balance engines: DVE does t-adds on even units
                        # (DVE 67->62us), PE does them on odd units (PE 53->62)
                        mode = "dve_t" if unit_idx % 2 == 0 else True
                    if mode == "dve_t":
                        # PE sums the 3 term products; DVE adds t_d from PSUM
                        # into SBUF (frees the PSUM bank immediately, one bank
                        # in flight per step, no ACT copy for z).
                        def step(d, vin, vout_sbuf):
                            ta = tpool.tile([P, 3, S, 2], F32, tag="ta")
                            tb = tpool.tile([P, 3, S, 2], F32, tag="tb")
                            tc_ = tpool.tile([P, 3, S, 2], F32, tag="tc")
                            vp = ppool.tile([P, 3, S, 2], F32, tag="v")
                            eng.tensor_mul(ta[:], x[:, :, :, :, d], _lead(vin[0], 0, 3))
                            eng.tensor_mul(tb[:], colr(d, 0), _lead(vin[1], 0, 3))
                            eng.tensor_mul(tc_[:], colr(d, 1), _lead(vin[2], 0, 3))
                            nc.tensor.matmul(vp[:], ident[:], ta[:],
                                             start=True, stop=False)
                            nc.tensor.matmul(vp[:], ident[:], tb[:],
                                             start=False, stop=False)
                            nc.tensor.matmul(vp[:], ident[:], tc_[:],
                                             start=False, stop=True)
                            eng.tensor_add(vout_sbuf[:], vp[:], colr(d, 2))

                        va = spool.tile([P, 3, S, 2], F32, tag="va")
                        vb = spool.tile([P, 3, S, 2], F32, tag="vb")
                        z = z_io.tile([P, 3, S, 2], F32, tag="z")
                        t3 = [m[:, :, :, 3, 3 * k + 2] for k in range(3)]
                        step(2, t3, va)
                        step(1, [va[:, k] for k in range(3)], vb)
                        step(0, [vb[:, k] for k in range(3)], z)
                    elif mode:
                        def step(d, vin, vout_psum):
                            ta = tpool.tile([P, 3, S, 2], F32, tag="ta")
                            tb = tpool.tile([P, 3, S, 2], F32, tag="tb")
                            tc_ = tpool.tile([P, 3, S, 2], F32, tag="tc")
                            eng.tensor_mul(ta[:], x[:, :, :, :, d], _lead(vin[0], 0, 3))
                            eng.tensor_mul(tb[:], colr(d, 0), _lead(vin[1], 0, 3))
                            eng.tensor_mul(tc_[:], colr(d, 1), _lead(vin[2], 0, 3))
                            nc.tensor.matmul(vout_psum[:], ident[:], ta[:],
                                             start=True, stop=False)
                            nc.tensor.matmul(vout_psum[:], ident[:], tb[:],
                                             start=False, stop=False)
                            nc.tensor.matmul(vout_psum[:], ident[:], tc_[:],
                                             start=False, stop=False)
                            nc.tensor.matmul(vout_psum[:], ident[:], colr(d, 2),
                                             start=False, stop=True)

                        va = ppool.tile([P, 3, S, 2], F32, tag="v")
                        vb = ppool.tile([P, 3, S, 2], F32, tag="v")
                        vz = ppool.tile([P, 3, S, 2], F32, tag="v")
                        t3 = [m[:, :, :, 3, 3 * k + 2] for k in range(3)]
                        step(2, t3, va)
                        step(1, [va[:, k] for k in range(3)], vb)
                        step(0, [vb[:, k] for k in range(3)], vz)
                        z = z_io.tile([P, 3, S, 2], F32, tag="z")
                        nc.scalar.copy(z[:], vz[:])
                    else:
                        va = spool.tile([P, 3, S, 2], F32, tag="va")
                        vb = spool.tile([P, 3, S, 2], F32, tag="vb")
                        tmp = spool.tile([P, 3, S, 2], F32, tag="tm")
                        z = z_io.tile([P, 3, S, 2], F32, tag="z")

                        def step(d, vin, vout):
                            eng.tensor_mul(vout[:], x[:, :, :, :, d], _lead(vin[0], 0, 3))
                            eng.tensor_mul(tmp[:], colr(d, 0), _lead(vin[1], 0, 3))
                            eng.tensor_add(vout[:], vout[:], tmp[:])
                            eng.tensor_mul(tmp[:], colr(d, 1), _lead(vin[2], 0, 3))
                            eng.tensor_add(vout[:], vout[:], tmp[:])
                            eng.tensor_add(vout[:], vout[:], colr(d, 2))

                        t3 = [m[:, :, :, 3, 3 * k + 2] for k in range(3)]
                        step(2, t3, va)
                        step(1, [va[:, k] for k in range(3)], vb)
                        step(0, [vb[:, k] for k in range(3)], z)

                    # z stores go out on the gpsimd SWDGE ring: a store queued
                    # on a HWDGE ring waits for DVE and head-of-line-blocks
                    # every input load queued behind it on that ring.
                    nc.gpsimd.dma_start(
                        out=zdst[:, 6 * col_base : 6 * (col_base + S)],
                        in_=z[:].rearrange("p r s c -> p (r s c)"),
                    )
                col_base += S
    nc.finalize()
    return nc


_NC_CACHE = {}


def _get_nc(cols=COLS, plan=DEFAULT_PLAN):
    key = (cols, plan)
    if key not in _NC_CACHE:
        _NC_CACHE[key] = build_nc(cols, plan)
    return _NC_CACHE[key]


def make_in_maps(output_pose, gt_pose):
    op = np.ascontiguousarray(output_pose, dtype=np.float32)
    gt = np.ascontiguousarray(gt_pose, dtype=np.float32)
    ident = np.eye(P, dtype=np.float32)
    return [
        {
            "output_pose": op[c * PER_CORE : (c + 1) * PER_CORE],
            "gt_pose": gt[c * PER_CORE : (c + 1) * PER_CORE],
            "ident": ident,
        }
        for c in range(N_CORES)
    ]


def run_device(output_pose, gt_pose, plan=DEFAULT_PLAN, trace=False):
    """Run the SPMD kernel; returns (results_list, BassKernelResults)."""
    nc = _get_nc(COLS, plan)
    in_maps = make_in_maps(output_pose, gt_pose)
    res = run_bass_kernel_spmd(nc, in_maps, list(range(N_CORES)), trace=trace)
    return res.results, res


def kernel(output_pose, gt_pose, gt_prev_pose=None, **_ignored):
    results, _ = run_device(output_pose, gt_pose)
    total = 0.0
    for r in results:
        d = r["z_out"].astype(np.float64) - r["z_gt"].astype(np.float64)
        total += float(np.sum(d * d))
    loss = np.float32(total / (B * 6))
    return (loss, loss)



# revision 2
# speedup vs baseline: 1.0902x; 1.0902x over previous
"""FK velocity loss kernel v2 for Trainium2 (8 NeuronCores, SPMD).

Key structure (vs the v1 baseline):
  * vel_loss == pos_loss exactly => gt_prev_pose never read.
  * All compute in fp16 on-chip: ScalarE converts fp32->fp16 into a
    TRANSPOSED layout (samples contiguous innermost), which makes every DVE
    operand stride-1 in its last dim => DVE high-rate mode.
  * Instructions fused across BOTH pose tensors and BOTH chains via a
    merged tc-axis of 4 (tensor-major, chain-minor) so every DVE op needs
    at most 3 free dims (4-dim non-mergeable APs crash the device).
  * Loss reduced on device: d = z_out - z_gt, then one tensor_tensor_reduce
    (d*d, sum) per lane into a per-lane accumulator column. Host sums
    acc[P, NL] across cores in float64. No z stores.
  * One-directional engine flow (rings -> ScalarE -> DVE) - no cross-engine
    feedback, so in-order queues never ping-pong.

Layout per lane (S samples/partition):
  m32[a]: [P, S*72] f32 raw DMA (tensor a), sample-major.
  m16:    [P, 4tc, 30, S] f16, tc = tensor*2 + chain; per chain: floats
          0..26 = depth-0..2 joints verbatim (f = 9d + 3r + k),
          27..29 = t3 (c2 of the depth-3 joint).
  x16:    [P, 4tc, 3r, 3d, S] f16 cross products.
  chain:  v <- R_d v + t_d for d=2,1,0 with v init t3; all on DVE.
"""

import numpy as np

import concourse.bass as bass
import concourse.bacc as bacc
import concourse.tile as tile
from concourse import mybir

B = 262144
N_CORES = 8
PER_CORE = B // N_CORES        # 32768
P = 128
COLS = PER_CORE // P           # 256 samples per partition
F32 = mybir.dt.float32
F16 = mybir.dt.float16

DEFAULT_PLAN = (64, 64, 64, 64)


def _ap(t, dims, offset=0):
    """AP over tile t with free dims [[stride,count],...] in elements."""
    base = t[:]
    return bass.AP(tensor=base.tensor, offset=base.offset + offset,
                   ap=[base.ap[0]] + [list(d) for d in dims])


def build_nc(plan=DEFAULT_PLAN, loop=None, stages="full", m16_bufs=2,
             x_bufs=1, dve_dtype=F16, pe_final=True, m32_bufs=4,
             v_bufs=1, s_bufs=1):
    assert sum(plan) == COLS
    NL = len(plan)
    NACC = 2 * NL if pe_final else NL
    per_core = COLS * P

    nc = bacc.Bacc()
    src_out = nc.declare_dram_parameter("output_pose", [per_core, 72], F32,
                                        isOutput=False)
    src_gt = nc.declare_dram_parameter("gt_pose", [per_core, 72], F32,
                                       isOutput=False)
    if pe_final:
        # [I | -I] fp16 stationaries for the PE d-accumulation
        identpm_in = nc.declare_dram_parameter("identpm", [P, 2 * P],
                                               mybir.dt.float16,
                                               isOutput=False)
    acc_out = nc.declare_dram_parameter("acc_out", [P, NACC], F32,
                                        isOutput=True)

    DT = dve_dtype

    import contextlib
    with tile.TileContext(nc) as tc:
        loop_ctx = tc.For_i(0, loop, 1) if loop else contextlib.nullcontext()
        with (
            loop_ctx,
            tc.tile_pool(name="m32_pool", bufs=m32_bufs) as m32pool,
            tc.tile_pool(name="m16_pool", bufs=m16_bufs) as m16pool,
            tc.tile_pool(name="x_pool", bufs=x_bufs) as xpool,
            tc.tile_pool(name="v_pool", bufs=v_bufs) as vpool,
            tc.tile_pool(name="s_pool", bufs=s_bufs) as spool,
            tc.tile_pool(name="acc_pool", bufs=1) as accpool,
            tc.tile_pool(name="psum_pool", bufs=4, space="PSUM") as ppool,
        ):
            acc = accpool.tile([P, NACC], F32)
            if pe_final:
                identpm = accpool.tile([P, 2 * P], mybir.dt.float16)
                nc.gpsimd.dma_start(out=identpm[:], in_=identpm_in[:])

            lanes = []
            col_base = 0
            for li, S in enumerate(plan):
                lanes.append((li, S, col_base))
                col_base += S

            # ---- Phase A: issue ALL lane loads up front (keeps both DMA
            # rings streaming; the Act-queue convs no longer gate ring B) ----
            lane_m32 = {}
            for li, S, cb in lanes:
                row0 = cb * P
                pair = []
                for a, (src, ring) in enumerate(
                    ((src_out, nc.sync), (src_gt, nc.scalar))
                ):
                    m32 = m32pool.tile([P, S * 72], F32, tag=f"m32_{a}",
                                       name=f"m32_{a}")
                    if stages != "compute":
                        ring.dma_start(
                            out=m32[:],
                            in_=src[row0: row0 + P * S, :].rearrange(
                                "(p s) f -> p (s f)", p=P),
                        )
                    pair.append(m32)
                lane_m32[li] = pair

            def emit_conv(li, S):
                # ScalarE fp32 -> fp16 transposed conversion.
                # m16 [P, 4tc, 30, S]; tensor a covers tc in {2a, 2a+1}.
                # out APs keep s innermost (unit stride); in APs may have
                # any inner stride (s steps by 72 in the raw layout).
                m32s = lane_m32[li]
                m16 = m16pool.tile([P, 4, 30, S], DT, tag="m16")
                for a in range(2):
                    toff = a * 60 * S
                    nc.scalar.copy(
                        _ap(m16, [[S, 27], [30 * S, 2], [1, S]], toff),
                        _ap(m32s[a], [[1, 27], [36, 2], [72, S]], 0),
                    )
                    nc.scalar.copy(
                        _ap(m16, [[S, 3], [30 * S, 2], [1, S]],
                            toff + 27 * S),
                        _ap(m32s[a], [[3, 3], [36, 2], [72, S]], 29),
                    )
                return m16

            def emit_compute(li, S, m16):
                def mcol(d, k):
                    """m16 column k of depth d: dims (tc4, r3, S)."""
                    return _ap(m16, [[30 * S, 4], [3 * S, 3], [1, S]],
                               (9 * d + k) * S)

                # DVE: cross products x = c0 x c1, depths 0..2
                # x16 [P, 4tc, 3r, 3d, S]
                x16 = xpool.tile([P, 4, 3, 3, S], DT, tag="x")
                tmp = spool.tile([P, 4, 3, S], DT, tag="tmp")
                for r in range(3):
                    r1, r2 = (r + 1) % 3, (r + 2) % 3
                    dims_in = [[30 * S, 4], [9 * S, 3], [1, S]]  # (tc, d, S)
                    xr = _ap(x16, [[9 * S, 4], [S, 3], [1, S]], r * 3 * S)
                    nc.vector.tensor_mul(
                        xr,
                        _ap(m16, dims_in, (3 * r1 + 0) * S),
                        _ap(m16, dims_in, (3 * r2 + 1) * S))
                    nc.vector.tensor_mul(
                        tmp[:],
                        _ap(m16, dims_in, (3 * r2 + 0) * S),
                        _ap(m16, dims_in, (3 * r1 + 1) * S))
                    nc.vector.tensor_sub(xr, xr, tmp[:])
                if stages == "cross":
                    return

                # DVE chain: v <- R_d v + t_d, d = 2, 1, 0
                # v tiles [P, 4tc, 3r, S]
                def vin_t3(j):
                    return _ap(m16, [[30 * S, 4], [0, 3], [1, S]],
                               (27 + j) * S)

                def vin_v(vt, j):
                    return _ap(vt, [[3 * S, 4], [0, 3], [1, S]], j * S)

                def xd(d):
                    return _ap(x16, [[9 * S, 4], [3 * S, 3], [1, S]], d * S)

                p0 = spool.tile([P, 4, 3, S], DT, tag="p0")
                p1 = spool.tile([P, 4, 3, S], DT, tag="p1")
                va = vpool.tile([P, 4, 3, S], DT, tag="va")
                vb = vpool.tile([P, 4, 3, S], DT, tag="vb")

                def step(d, vin, vout):
                    nc.vector.tensor_mul(p0[:], xd(d), vin(0))
                    nc.vector.tensor_mul(p1[:], mcol(d, 0), vin(1))
                    nc.vector.tensor_add(p0[:], p0[:], p1[:])
                    nc.vector.tensor_mul(p1[:], mcol(d, 1), vin(2))
                    nc.vector.tensor_add(p0[:], p0[:], p1[:])
                    nc.vector.tensor_add(vout[:], p0[:], mcol(d, 2))

                step(2, vin_t3, va)
                step(1, lambda j: vin_v(va, j), vb)

                if not pe_final:
                    z = vpool.tile([P, 4, 3, S], DT, tag="z")
                    step(0, lambda j: vin_v(vb, j), z)
                    # loss: acc[:, li] = sum of (z0 - z1)^2
                    # z [P, 4tc, 3r, S]: out half tc in {0,1}, gt {2,3}
                    d16 = spool.tile([P, 2, 3, S], DT, tag="d16")
                    dsq = spool.tile([P, 2, 3, S], F32, tag="dsq")
                    zdims = [[3 * S, 2], [S, 3], [1, S]]
                    nc.vector.tensor_sub(d16[:], _ap(z, zdims, 0),
                                         _ap(z, zdims, 6 * S))
                    nc.scalar.activation(
                        dsq[:], d16[:], mybir.ActivationFunctionType.Square,
                        accum_out=acc[:, li: li + 1])
                else:
                    # step 0 products on DVE; d = z_out - z_gt accumulated
                    # directly in PSUM via [I | -I] stationaries.
                    px = spool.tile([P, 4, 3, S], DT, tag="px")
                    pc0 = spool.tile([P, 4, 3, S], DT, tag="pc0")
                    pc1 = spool.tile([P, 4, 3, S], DT, tag="pc1")
                    nc.vector.tensor_mul(px[:], xd(0), vin_v(vb, 0))
                    nc.vector.tensor_mul(pc0[:], mcol(0, 0), vin_v(vb, 1))
                    nc.vector.tensor_mul(pc1[:], mcol(0, 1), vin_v(vb, 2))
                    dsq = spool.tile([P, 3, S], F32, tag="dsq")
                    for c in range(2):
                        pd = ppool.tile([P, 3, S], F32, tag=f"pd{c}",
                                        name=f"pd{c}")
                        first = True
                        for a in range(2):
                            tcix = 2 * a + c
                            stat = identpm[:, a * P:(a + 1) * P]
                            movs = [
                                _ap(px, [[S, 3], [1, S]], tcix * 3 * S),
                                _ap(pc0, [[S, 3], [1, S]], tcix * 3 * S),
                                _ap(pc1, [[S, 3], [1, S]], tcix * 3 * S),
                                _ap(m16, [[3 * S, 3], [1, S]],
                                    tcix * 30 * S + 2 * S),
                            ]
                            for mi, mov in enumerate(movs):
                                nc.tensor.matmul(
                                    pd[:], stat, mov, start=first,
                                    stop=(a == 1 and mi == 3))
                                first = False
                        nc.scalar.activation(
                            dsq[:], pd[:],
                            mybir.ActivationFunctionType.Square,
                            accum_out=acc[:, 2 * li + c: 2 * li + c + 1])

            # ---- Phase B/C: conv skewed one lane ahead of compute ----
            if stages not in ("dma",):
                lane_m16 = {0: emit_conv(0, plan[0])}
                for li, S, cb in lanes:
                    if li + 1 < NL:
                        lane_m16[li + 1] = emit_conv(li + 1, plan[li + 1])
                    if stages != "conv":
                        emit_compute(li, S, lane_m16[li])

            if stages == "full":
                nc.gpsimd.dma_start(out=acc_out[:], in_=acc[:])
    nc.finalize()
    return nc


_NC_CACHE = {}


def _get_nc():
    if 'nc' not in _NC_CACHE:
        _NC_CACHE['nc'] = build_nc()
    return _NC_CACHE['nc']


def make_in_maps(output_pose, gt_pose, pe_final=True):
    op = np.ascontiguousarray(output_pose, dtype=np.float32)
    gt = np.ascontiguousarray(gt_pose, dtype=np.float32)
    maps = [
        {
            "output_pose": op[c * PER_CORE: (c + 1) * PER_CORE],
            "gt_pose": gt[c * PER_CORE: (c + 1) * PER_CORE],
        }
        for c in range(N_CORES)
    ]
    if pe_final:
        ident = np.eye(P, dtype=np.float16)
        identpm = np.concatenate([ident, -ident], axis=1)
        for m in maps:
            m["identpm"] = identpm
    return maps


def kernel(output_pose, gt_pose, gt_prev_pose=None, **_ignored):
    from concourse.bass_utils import run_bass_kernel_spmd
    nc = _get_nc()
    in_maps = make_in_maps(output_pose, gt_pose)
    res = run_bass_kernel_spmd(nc, in_maps, list(range(N_CORES)))
    total = 0.0
    for r in res.results:
        total += float(np.sum(r["acc_out"].astype(np.float64)))
    loss = np.float32(total / (B * 6))
    return (loss, loss)


# revision 3
# speedup vs baseline: 1.2624x; 1.1579x over previous
"""FK velocity loss kernel v2 for Trainium2 (8 NeuronCores, SPMD).

Key structure (vs the v1 baseline):
  * vel_loss == pos_loss exactly => gt_prev_pose never read.
  * All compute in fp16 on-chip: ScalarE converts fp32->fp16 into a
    TRANSPOSED layout (samples contiguous innermost), which makes every DVE
    operand stride-1 in its last dim => DVE high-rate mode.
  * Instructions fused across BOTH pose tensors and BOTH chains via a
    merged tc-axis of 4 (tensor-major, chain-minor) so every DVE op needs
    at most 3 free dims (4-dim non-mergeable APs crash the device).
  * Loss reduced on device: d = z_out - z_gt, then one tensor_tensor_reduce
    (d*d, sum) per lane into a per-lane accumulator column. Host sums
    acc[P, NL] across cores in float64. No z stores.
  * One-directional engine flow (rings -> ScalarE -> DVE) - no cross-engine
    feedback, so in-order queues never ping-pong.

Layout per lane (S samples/partition):
  m32[a]: [P, S*72] f32 raw DMA (tensor a), sample-major.
  m16:    [P, 4tc, 30, S] f16, tc = tensor*2 + chain; per chain: floats
          0..26 = depth-0..2 joints verbatim (f = 9d + 3r + k),
          27..29 = t3 (c2 of the depth-3 joint).
  x16:    [P, 4tc, 3r, 3d, S] f16 cross products.
  chain:  v <- R_d v + t_d for d=2,1,0 with v init t3; all on DVE.
"""

import numpy as np

import concourse.bass as bass
import concourse.bacc as bacc
import concourse.tile as tile
from concourse import mybir

B = 262144
N_CORES = 8
PER_CORE = B // N_CORES        # 32768
P = 128
COLS = PER_CORE // P           # 256 samples per partition
F32 = mybir.dt.float32
F16 = mybir.dt.float16

DEFAULT_PLAN = (64, 64, 64, 64)


def _ap(t, dims, offset=0):
    """AP over tile t with free dims [[stride,count],...] in elements."""
    base = t[:]
    return bass.AP(tensor=base.tensor, offset=base.offset + offset,
                   ap=[base.ap[0]] + [list(d) for d in dims])


def build_nc(plan=DEFAULT_PLAN, loop=None, stages="full", m16_bufs=2,
             x_bufs=1, dve_dtype=F16, pe_final=True, m32_bufs=2,
             v_bufs=1, s_bufs=1):
    assert sum(plan) == COLS
    NL = len(plan)
    NACC = 2 * NL if pe_final else NL
    per_core = COLS * P

    nc = bacc.Bacc()
    src_out = nc.declare_dram_parameter("output_pose", [per_core, 72], F32,
                                        isOutput=False)
    src_gt = nc.declare_dram_parameter("gt_pose", [per_core, 72], F32,
                                       isOutput=False)
    if pe_final:
        # [I | -I] fp16 stationaries for the PE d-accumulation
        identpm_in = nc.declare_dram_parameter("identpm", [P, 2 * P],
                                               mybir.dt.float16,
                                               isOutput=False)
    acc_out = nc.declare_dram_parameter("acc_out", [P, NACC], F32,
                                        isOutput=True)

    DT = dve_dtype

    import contextlib
    with tile.TileContext(nc) as tc:
        loop_ctx = tc.For_i(0, loop, 1) if loop else contextlib.nullcontext()
        with (
            loop_ctx,
            tc.tile_pool(name="m32_pool", bufs=m32_bufs) as m32pool,
            tc.tile_pool(name="m16_pool", bufs=m16_bufs) as m16pool,
            tc.tile_pool(name="x_pool", bufs=x_bufs) as xpool,
            tc.tile_pool(name="v_pool", bufs=v_bufs) as vpool,
            tc.tile_pool(name="s_pool", bufs=s_bufs) as spool,
            tc.tile_pool(name="acc_pool", bufs=1) as accpool,
            tc.tile_pool(name="psum_pool", bufs=4, space="PSUM") as ppool,
        ):
            acc = accpool.tile([P, NACC], F32)
            if pe_final:
                identpm = accpool.tile([P, 2 * P], mybir.dt.float16)
                nc.gpsimd.dma_start(out=identpm[:], in_=identpm_in[:])

            lanes = []
            col_base = 0
            for li, S in enumerate(plan):
                lanes.append((li, S, col_base))
                col_base += S

            # ---- DMA issue: stagger K lanes ahead. The HWDGE ring has 16
            # channels and round-robins queued transfers, so issuing ALL
            # lanes up front makes every lane finish together (no early
            # data, no overlap). K in flight => lane l lands ~K transfers
            # after its issue while the ring stays fed. ----
            lane_m32 = {}

            def issue_dma(li):
                S = plan[li]
                cb = sum(plan[:li])
                row0 = cb * P
                pair = []
                for a, (src, ring) in enumerate(
                    ((src_out, nc.sync), (src_gt, nc.scalar))
                ):
                    m32 = m32pool.tile([P, S * 72], F32, tag=f"m32_{a}",
                                       name=f"m32_{a}")
                    if stages != "compute":
                        ring.dma_start(
                            out=m32[:],
                            in_=src[row0: row0 + P * S, :].rearrange(
                                "(p s) f -> p (s f)", p=P),
                        )
                    pair.append(m32)
                lane_m32[li] = pair

            DMA_AHEAD = min(2, NL)
            for li in range(DMA_AHEAD):
                issue_dma(li)

            def emit_conv(li, S):
                # ScalarE fp32 -> fp16 transposed conversion.
                # m16 [P, 4tc, 30, S]; tensor a covers tc in {2a, 2a+1}.
                # out APs keep s innermost (unit stride); in APs may have
                # any inner stride (s steps by 72 in the raw layout).
                m32s = lane_m32[li]
                m16 = m16pool.tile([P, 4, 30, S], DT, tag="m16")
                for a in range(2):
                    toff = a * 60 * S
                    nc.scalar.copy(
                        _ap(m16, [[S, 27], [30 * S, 2], [1, S]], toff),
                        _ap(m32s[a], [[1, 27], [36, 2], [72, S]], 0),
                    )
                    nc.scalar.copy(
                        _ap(m16, [[S, 3], [30 * S, 2], [1, S]],
                            toff + 27 * S),
                        _ap(m32s[a], [[3, 3], [36, 2], [72, S]], 29),
                    )
                return m16

            def emit_compute(li, S, m16):
                def mcol(d, k):
                    """m16 column k of depth d: dims (tc4, r3, S)."""
                    return _ap(m16, [[30 * S, 4], [3 * S, 3], [1, S]],
                               (9 * d + k) * S)

                # DVE: cross products x = c0 x c1, depths 0..2
                # x16 [P, 4tc, 3r, 3d, S]
                x16 = xpool.tile([P, 4, 3, 3, S], DT, tag="x")
                tmp = spool.tile([P, 4, 3, S], DT, tag="tmp")
                for r in range(3):
                    r1, r2 = (r + 1) % 3, (r + 2) % 3
                    dims_in = [[30 * S, 4], [9 * S, 3], [1, S]]  # (tc, d, S)
                    xr = _ap(x16, [[9 * S, 4], [S, 3], [1, S]], r * 3 * S)
                    nc.vector.tensor_mul(
                        xr,
                        _ap(m16, dims_in, (3 * r1 + 0) * S),
                        _ap(m16, dims_in, (3 * r2 + 1) * S))
                    nc.vector.tensor_mul(
                        tmp[:],
                        _ap(m16, dims_in, (3 * r2 + 0) * S),
                        _ap(m16, dims_in, (3 * r1 + 1) * S))
                    nc.vector.tensor_sub(xr, xr, tmp[:])
                if stages == "cross":
                    return

                # DVE chain: v <- R_d v + t_d, d = 2, 1, 0
                # v tiles [P, 4tc, 3r, S]
                def vin_t3(j):
                    return _ap(m16, [[30 * S, 4], [0, 3], [1, S]],
                               (27 + j) * S)

                def vin_v(vt, j):
                    return _ap(vt, [[3 * S, 4], [0, 3], [1, S]], j * S)

                def xd(d):
                    return _ap(x16, [[9 * S, 4], [3 * S, 3], [1, S]], d * S)

                p0 = spool.tile([P, 4, 3, S], DT, tag="p0")
                p1 = spool.tile([P, 4, 3, S], DT, tag="p1")
                va = vpool.tile([P, 4, 3, S], DT, tag="va")
                vb = vpool.tile([P, 4, 3, S], DT, tag="vb")

                def step(d, vin, vout):
                    nc.vector.tensor_mul(p0[:], xd(d), vin(0))
                    nc.vector.tensor_mul(p1[:], mcol(d, 0), vin(1))
                    nc.vector.tensor_add(p0[:], p0[:], p1[:])
                    nc.vector.tensor_mul(p1[:], mcol(d, 1), vin(2))
                    nc.vector.tensor_add(p0[:], p0[:], p1[:])
                    nc.vector.tensor_add(vout[:], p0[:], mcol(d, 2))

                step(2, vin_t3, va)
                step(1, lambda j: vin_v(va, j), vb)

                if not pe_final:
                    z = vpool.tile([P, 4, 3, S], DT, tag="z")
                    step(0, lambda j: vin_v(vb, j), z)
                    # loss: acc[:, li] = sum of (z0 - z1)^2
                    # z [P, 4tc, 3r, S]: out half tc in {0,1}, gt {2,3}
                    d16 = spool.tile([P, 2, 3, S], DT, tag="d16")
                    dsq = spool.tile([P, 2, 3, S], F32, tag="dsq")
                    zdims = [[3 * S, 2], [S, 3], [1, S]]
                    nc.vector.tensor_sub(d16[:], _ap(z, zdims, 0),
                                         _ap(z, zdims, 6 * S))
                    nc.scalar.activation(
                        dsq[:], d16[:], mybir.ActivationFunctionType.Square,
                        accum_out=acc[:, li: li + 1])
                else:
                    # step 0 products on DVE; d = z_out - z_gt accumulated
                    # directly in PSUM via [I | -I] stationaries.
                    px = spool.tile([P, 4, 3, S], DT, tag="px")
                    pc0 = spool.tile([P, 4, 3, S], DT, tag="pc0")
                    pc1 = spool.tile([P, 4, 3, S], DT, tag="pc1")
                    nc.vector.tensor_mul(px[:], xd(0), vin_v(vb, 0))
                    nc.vector.tensor_mul(pc0[:], mcol(0, 0), vin_v(vb, 1))
                    nc.vector.tensor_mul(pc1[:], mcol(0, 1), vin_v(vb, 2))
                    for c in range(2):
                        pd = ppool.tile([P, 3, S], F32, tag=f"pd{li}_{c}",
                                        bufs=1, name=f"pd{li}_{c}")
                        first = True
                        for a in range(2):
                            tcix = 2 * a + c
                            stat = identpm[:, a * P:(a + 1) * P]
                            movs = [
                                _ap(px, [[S, 3], [1, S]], tcix * 3 * S),
                                _ap(pc0, [[S, 3], [1, S]], tcix * 3 * S),
                                _ap(pc1, [[S, 3], [1, S]], tcix * 3 * S),
                                _ap(m16, [[3 * S, 3], [1, S]],
                                    tcix * 30 * S + 2 * S),
                            ]
                            for mi, mov in enumerate(movs):
                                nc.tensor.matmul(
                                    pd[:], stat, mov, start=first,
                                    stop=(a == 1 and mi == 3))
                                first = False
                        pending_sq.append((pd, 2 * li + c))

            # ---- Phase B/C: conv skewed one lane ahead of compute;
            # squares deferred to the end so the in-order Act queue never
            # makes a later conv wait on an earlier lane's DVE+PE chain ----
            pending_sq = []
            if stages == "dma":
                for li in range(DMA_AHEAD, NL):
                    issue_dma(li)
            else:
                lane_m16 = {0: emit_conv(0, plan[0])}
                for li, S, cb in lanes:
                    if li + DMA_AHEAD < NL:
                        issue_dma(li + DMA_AHEAD)
                    if li + 1 < NL:
                        lane_m16[li + 1] = emit_conv(li + 1, plan[li + 1])
                    if stages != "conv":
                        emit_compute(li, S, lane_m16[li])
                Smax = max(plan)
                dsq = spool.tile([P, 3, Smax], F32, tag="dsq")
                for pd, col in pending_sq:
                    nc.scalar.activation(
                        dsq[:, :, 0:pd.shape[-1]], pd[:],
                        mybir.ActivationFunctionType.Square,
                        accum_out=acc[:, col: col + 1])

            if stages == "full":
                nc.gpsimd.dma_start(out=acc_out[:], in_=acc[:])
    nc.finalize()
    return nc


_NC_CACHE = {}


def _get_nc():
    if 'nc' not in _NC_CACHE:
        _NC_CACHE['nc'] = build_nc()
    return _NC_CACHE['nc']


def make_in_maps(output_pose, gt_pose, pe_final=True):
    op = np.ascontiguousarray(output_pose, dtype=np.float32)
    gt = np.ascontiguousarray(gt_pose, dtype=np.float32)
    maps = [
        {
            "output_pose": op[c * PER_CORE: (c + 1) * PER_CORE],
            "gt_pose": gt[c * PER_CORE: (c + 1) * PER_CORE],
        }
        for c in range(N_CORES)
    ]
    if pe_final:
        ident = np.eye(P, dtype=np.float16)
        identpm = np.concatenate([ident, -ident], axis=1)
        for m in maps:
            m["identpm"] = identpm
    return maps


def kernel(output_pose, gt_pose, gt_prev_pose=None, **_ignored):
    from concourse.bass_utils import run_bass_kernel_spmd
    nc = _get_nc()
    in_maps = make_in_maps(output_pose, gt_pose)
    res = run_bass_kernel_spmd(nc, in_maps, list(range(N_CORES)))
    total = 0.0
    for r in res.results:
        total += float(np.sum(r["acc_out"].astype(np.float64)))
    loss = np.float32(total / (B * 6))
    return (loss, loss)


# revision 4
# speedup vs baseline: 1.3128x; 1.0399x over previous
"""FK velocity loss kernel v2 for Trainium2 (8 NeuronCores, SPMD).

Key structure (vs the v1 baseline):
  * vel_loss == pos_loss exactly => gt_prev_pose never read.
  * All compute in fp16 on-chip: ScalarE converts fp32->fp16 into a
    TRANSPOSED layout (samples contiguous innermost), which makes every DVE
    operand stride-1 in its last dim => DVE high-rate mode.
  * Instructions fused across BOTH pose tensors and BOTH chains via a
    merged tc-axis of 4 (tensor-major, chain-minor) so every DVE op needs
    at most 3 free dims (4-dim non-mergeable APs crash the device).
  * Loss reduced on device: d = z_out - z_gt, then one tensor_tensor_reduce
    (d*d, sum) per lane into a per-lane accumulator column. Host sums
    acc[P, NL] across cores in float64. No z stores.
  * One-directional engine flow (rings -> ScalarE -> DVE) - no cross-engine
    feedback, so in-order queues never ping-pong.

Layout per lane (S samples/partition):
  m32[a]: [P, S*72] f32 raw DMA (tensor a), sample-major.
  m16:    [P, 4tc, 30, S] f16, tc = tensor*2 + chain; per chain: floats
          0..26 = depth-0..2 joints verbatim (f = 9d + 3r + k),
          27..29 = t3 (c2 of the depth-3 joint).
  x16:    [P, 4tc, 3r, 3d, S] f16 cross products.
  chain:  v <- R_d v + t_d for d=2,1,0 with v init t3; all on DVE.
"""

import numpy as np

import concourse.bass as bass
import concourse.bacc as bacc
import concourse.tile as tile
from concourse import mybir

B = 262144
N_CORES = 8
PER_CORE = B // N_CORES        # 32768
P = 128
COLS = PER_CORE // P           # 256 samples per partition
F32 = mybir.dt.float32
F16 = mybir.dt.float16

DEFAULT_PLAN = (32,) * 8


def _ap(t, dims, offset=0):
    """AP over tile t with free dims [[stride,count],...] in elements."""
    base = t[:]
    return bass.AP(tensor=base.tensor, offset=base.offset + offset,
                   ap=[base.ap[0]] + [list(d) for d in dims])


def build_nc(plan=DEFAULT_PLAN, loop=None, stages="full", m16_bufs=2,
             x_bufs=1, dve_dtype=F16, pe_final=True, m32_bufs=2,
             v_bufs=1, s_bufs=1, swdge_frac=0.0, dma_ahead=3):
    assert sum(plan) == COLS
    NL = len(plan)
    NACC = 2 * NL if pe_final else NL
    per_core = COLS * P

    nc = bacc.Bacc()
    src_out = nc.declare_dram_parameter("output_pose", [per_core, 72], F32,
                                        isOutput=False)
    src_gt = nc.declare_dram_parameter("gt_pose", [per_core, 72], F32,
                                       isOutput=False)
    if pe_final:
        # [I | -I] fp16 stationaries for the PE d-accumulation
        identpm_in = nc.declare_dram_parameter("identpm", [P, 2 * P],
                                               mybir.dt.float16,
                                               isOutput=False)
    acc_out = nc.declare_dram_parameter("acc_out", [P, NACC], F32,
                                        isOutput=True)

    DT = dve_dtype

    import contextlib
    with tile.TileContext(nc) as tc:
        loop_ctx = tc.For_i(0, loop, 1) if loop else contextlib.nullcontext()
        with (
            loop_ctx,
            tc.tile_pool(name="m32_pool", bufs=m32_bufs) as m32pool,
            tc.tile_pool(name="m16_pool", bufs=m16_bufs) as m16pool,
            tc.tile_pool(name="x_pool", bufs=x_bufs) as xpool,
            tc.tile_pool(name="v_pool", bufs=v_bufs) as vpool,
            tc.tile_pool(name="s_pool", bufs=s_bufs) as spool,
            tc.tile_pool(name="acc_pool", bufs=1) as accpool,
            tc.tile_pool(name="psum_pool", bufs=4, space="PSUM") as ppool,
        ):
            acc = accpool.tile([P, NACC], F32)
            if pe_final:
                identpm = accpool.tile([P, 2 * P], mybir.dt.float16)
                nc.gpsimd.dma_start(out=identpm[:], in_=identpm_in[:])

            lanes = []
            col_base = 0
            for li, S in enumerate(plan):
                lanes.append((li, S, col_base))
                col_base += S

            # ---- DMA issue: stagger K lanes ahead. The HWDGE ring has 16
            # channels and round-robins queued transfers, so issuing ALL
            # lanes up front makes every lane finish together (no early
            # data, no overlap). K in flight => lane l lands ~K transfers
            # after its issue while the ring stays fed. ----
            lane_m32 = {}

            def issue_dma(li):
                S = plan[li]
                cb = sum(plan[:li])
                row0 = cb * P
                pair = []
                for a, (src, ring) in enumerate(
                    ((src_out, nc.sync), (src_gt, nc.scalar))
                ):
                    m32 = m32pool.tile([P, S * 72], F32, tag=f"m32_{a}",
                                       name=f"m32_{a}")
                    if stages != "compute":
                        srcv = src[row0: row0 + P * S, :].rearrange(
                            "(p s) f -> p (s f)", p=P)
                        if swdge_frac > 0.0:
                            # offload the tail of each load to the otherwise
                            # idle gpsimd SWDGE ring (~170 GB/s measured)
                            h = int(S * (1.0 - swdge_frac)) * 72
                            ring.dma_start(out=m32[:, :h], in_=srcv[:, :h])
                            nc.gpsimd.dma_start(out=m32[:, h:],
                                                in_=srcv[:, h:])
                        else:
                            ring.dma_start(out=m32[:], in_=srcv)
                    pair.append(m32)
                lane_m32[li] = pair

            DMA_AHEAD = min(dma_ahead, NL)
            for li in range(DMA_AHEAD):
                issue_dma(li)

            def emit_conv(li, S):
                # ScalarE fp32 -> fp16 transposed conversion.
                # m16 [P, 4tc, 30, S]; tensor a covers tc in {2a, 2a+1}.
                # out APs keep s innermost (unit stride); in APs may have
                # any inner stride (s steps by 72 in the raw layout).
                m32s = lane_m32[li]
                m16 = m16pool.tile([P, 4, 30, S], DT, tag="m16")
                for a in range(2):
                    toff = a * 60 * S
                    nc.scalar.copy(
                        _ap(m16, [[S, 27], [30 * S, 2], [1, S]], toff),
                        _ap(m32s[a], [[1, 27], [36, 2], [72, S]], 0),
                    )
                    nc.scalar.copy(
                        _ap(m16, [[S, 3], [30 * S, 2], [1, S]],
                            toff + 27 * S),
                        _ap(m32s[a], [[3, 3], [36, 2], [72, S]], 29),
                    )
                return m16

            def emit_compute(li, S, m16):
                def mcol(d, k):
                    """m16 column k of depth d: dims (tc4, r3, S)."""
                    return _ap(m16, [[30 * S, 4], [3 * S, 3], [1, S]],
                               (9 * d + k) * S)

                # DVE: cross products x = c0 x c1, depths 0..2
                # x16 [P, 4tc, 3r, 3d, S]
                x16 = xpool.tile([P, 4, 3, 3, S], DT, tag="x")
                tmp = spool.tile([P, 4, 3, S], DT, tag="tmp")
                for r in range(3):
                    r1, r2 = (r + 1) % 3, (r + 2) % 3
                    dims_in = [[30 * S, 4], [9 * S, 3], [1, S]]  # (tc, d, S)
                    xr = _ap(x16, [[9 * S, 4], [S, 3], [1, S]], r * 3 * S)
                    nc.vector.tensor_mul(
                        xr,
                        _ap(m16, dims_in, (3 * r1 + 0) * S),
                        _ap(m16, dims_in, (3 * r2 + 1) * S))
                    nc.vector.tensor_mul(
                        tmp[:],
                        _ap(m16, dims_in, (3 * r2 + 0) * S),
                        _ap(m16, dims_in, (3 * r1 + 1) * S))
                    nc.vector.tensor_sub(xr, xr, tmp[:])
                if stages == "cross":
                    return

                # DVE chain: v <- R_d v + t_d, d = 2, 1, 0
                # v tiles [P, 4tc, 3r, S]
                def vin_t3(j):
                    return _ap(m16, [[30 * S, 4], [0, 3], [1, S]],
                               (27 + j) * S)

                def vin_v(vt, j):
                    return _ap(vt, [[3 * S, 4], [0, 3], [1, S]], j * S)

                def xd(d):
                    return _ap(x16, [[9 * S, 4], [3 * S, 3], [1, S]], d * S)

                p0 = spool.tile([P, 4, 3, S], DT, tag="p0")
                p1 = spool.tile([P, 4, 3, S], DT, tag="p1")
                va = vpool.tile([P, 4, 3, S], DT, tag="va")
                vb = vpool.tile([P, 4, 3, S], DT, tag="vb")

                def step(d, vin, vout):
                    nc.vector.tensor_mul(p0[:], xd(d), vin(0))
                    nc.vector.tensor_mul(p1[:], mcol(d, 0), vin(1))
                    nc.vector.tensor_add(p0[:], p0[:], p1[:])
                    nc.vector.tensor_mul(p1[:], mcol(d, 1), vin(2))
                    nc.vector.tensor_add(p0[:], p0[:], p1[:])
                    nc.vector.tensor_add(vout[:], p0[:], mcol(d, 2))

                step(2, vin_t3, va)
                step(1, lambda j: vin_v(va, j), vb)

                if not pe_final:
                    z = vpool.tile([P, 4, 3, S], DT, tag="z")
                    step(0, lambda j: vin_v(vb, j), z)
                    # loss: acc[:, li] = sum of (z0 - z1)^2
                    # z [P, 4tc, 3r, S]: out half tc in {0,1}, gt {2,3}
                    d16 = spool.tile([P, 2, 3, S], DT, tag="d16")
                    dsq = spool.tile([P, 2, 3, S], F32, tag="dsq")
                    zdims = [[3 * S, 2], [S, 3], [1, S]]
                    nc.vector.tensor_sub(d16[:], _ap(z, zdims, 0),
                                         _ap(z, zdims, 6 * S))
                    nc.scalar.activation(
                        dsq[:], d16[:], mybir.ActivationFunctionType.Square,
                        accum_out=acc[:, li: li + 1])
                else:
                    # step 0 products on DVE; d = z_out - z_gt accumulated
                    # directly in PSUM via [I | -I] stationaries.
                    px = spool.tile([P, 4, 3, S], DT, tag="px")
                    pc0 = spool.tile([P, 4, 3, S], DT, tag="pc0")
                    pc1 = spool.tile([P, 4, 3, S], DT, tag="pc1")
                    nc.vector.tensor_mul(px[:], xd(0), vin_v(vb, 0))
                    nc.vector.tensor_mul(pc0[:], mcol(0, 0), vin_v(vb, 1))
                    nc.vector.tensor_mul(pc1[:], mcol(0, 1), vin_v(vb, 2))
                    for c in range(2):
                        pd = ppool.tile([P, 3, S], F32, tag=f"pd{li % 4}_{c}",
                                        bufs=1, name=f"pd{li}_{c}")
                        first = True
                        for a in range(2):
                            tcix = 2 * a + c
                            stat = identpm[:, a * P:(a + 1) * P]
                            movs = [
                                _ap(px, [[S, 3], [1, S]], tcix * 3 * S),
                                _ap(pc0, [[S, 3], [1, S]], tcix * 3 * S),
                                _ap(pc1, [[S, 3], [1, S]], tcix * 3 * S),
                                _ap(m16, [[3 * S, 3], [1, S]],
                                    tcix * 30 * S + 2 * S),
                            ]
                            for mi, mov in enumerate(movs):
                                nc.tensor.matmul(
                                    pd[:], stat, mov, start=first,
                                    stop=(a == 1 and mi == 3))
                                first = False
                        pending_sq.append((pd, 2 * li + c))

            # ---- Phase B/C: conv skewed one lane ahead of compute;
            # squares deferred to the end so the in-order Act queue never
            # makes a later conv wait on an earlier lane's DVE+PE chain ----
            pending_sq = []
            if stages == "dma":
                for li in range(DMA_AHEAD, NL):
                    issue_dma(li)
            else:
                lane_m16 = {0: emit_conv(0, plan[0])}
                Smax = max(plan)
                dsq = spool.tile([P, 3, Smax], F32, tag="dsq")

                def drain_squares():
                    for pd, col in pending_sq:
                        nc.scalar.activation(
                            dsq[:, :, 0:pd.shape[-1]], pd[:],
                            mybir.ActivationFunctionType.Square,
                            accum_out=acc[:, col: col + 1])
                    pending_sq.clear()

                for li, S, cb in lanes:
                    if li + DMA_AHEAD < NL:
                        issue_dma(li + DMA_AHEAD)
                    if li + 1 < NL:
                        lane_m16[li + 1] = emit_conv(li + 1, plan[li + 1])
                    if stages != "conv":
                        emit_compute(li, S, lane_m16[li])
                    if len(pending_sq) >= 8:
                        drain_squares()
                drain_squares()

            if stages == "full":
                nc.gpsimd.dma_start(out=acc_out[:], in_=acc[:])
    nc.finalize()
    return nc


_NC_CACHE = {}


def _get_nc():
    if 'nc' not in _NC_CACHE:
        _NC_CACHE['nc'] = build_nc()
    return _NC_CACHE['nc']


def make_in_maps(output_pose, gt_pose, pe_final=True):
    op = np.ascontiguousarray(output_pose, dtype=np.float32)
    gt = np.ascontiguousarray(gt_pose, dtype=np.float32)
    maps = [
        {
            "output_pose": op[c * PER_CORE: (c + 1) * PER_CORE],
            "gt_pose": gt[c * PER_CORE: (c + 1) * PER_CORE],
        }
        for c in range(N_CORES)
    ]
    if pe_final:
        ident = np.eye(P, dtype=np.float16)
        identpm = np.concatenate([ident, -ident], axis=1)
        for m in maps:
            m["identpm"] = identpm
    return maps


def kernel(output_pose, gt_pose, gt_prev_pose=None, **_ignored):
    from concourse.bass_utils import run_bass_kernel_spmd
    nc = _get_nc()
    in_maps = make_in_maps(output_pose, gt_pose)
    res = run_bass_kernel_spmd(nc, in_maps, list(range(N_CORES)))
    total = 0.0
    for r in res.results:
        total += float(np.sum(r["acc_out"].astype(np.float64)))
    loss = np.float32(total / (B * 6))
    return (loss, loss)


# revision 5
# speedup vs baseline: 1.5657x; 1.1927x over previous
"""FK velocity loss kernel v2 for Trainium2 (8 NeuronCores, SPMD).

Key structure (vs the v1 baseline):
  * vel_loss == pos_loss exactly => gt_prev_pose never read.
  * All compute in fp16 on-chip: ScalarE converts fp32->fp16 into a
    TRANSPOSED layout (samples contiguous innermost), which makes every DVE
    operand stride-1 in its last dim => DVE high-rate mode.
  * Instructions fused across BOTH pose tensors and BOTH chains via a
    merged tc-axis of 4 (tensor-major, chain-minor) so every DVE op needs
    at most 3 free dims (4-dim non-mergeable APs crash the device).
  * Loss reduced on device: d = z_out - z_gt, then one tensor_tensor_reduce
    (d*d, sum) per lane into a per-lane accumulator column. Host sums
    acc[P, NL] across cores in float64. No z stores.
  * One-directional engine flow (rings -> ScalarE -> DVE) - no cross-engine
    feedback, so in-order queues never ping-pong.

Layout per lane (S samples/partition):
  m32[a]: [P, S*72] f32 raw DMA (tensor a), sample-major.
  m16:    [P, 4tc, 30, S] f16, tc = tensor*2 + chain; per chain: floats
          0..26 = depth-0..2 joints verbatim (f = 9d + 3r + k),
          27..29 = t3 (c2 of the depth-3 joint).
  x16:    [P, 4tc, 3r, 3d, S] f16 cross products.
  chain:  v <- R_d v + t_d for d=2,1,0 with v init t3; all on DVE.
"""

import numpy as np

import concourse.bass as bass
import concourse.bacc as bacc
import concourse.tile as tile
from concourse import mybir

B = 262144
N_CORES = 8
PER_CORE = B // N_CORES        # 32768
P = 128
COLS = PER_CORE // P           # 256 samples per partition
F32 = mybir.dt.float32
F16 = mybir.dt.float16

DEFAULT_PLAN = (32,) * 8


def _ap(t, dims, offset=0):
    """AP over tile t with free dims [[stride,count],...] in elements."""
    base = t[:]
    return bass.AP(tensor=base.tensor, offset=base.offset + offset,
                   ap=[base.ap[0]] + [list(d) for d in dims])


def build_nc(plan=DEFAULT_PLAN, loop=None, stages="full", m16_bufs=3,
             x_bufs=1, dve_dtype=F16, pe_final=True, m32_bufs=2,
             v_bufs=1, s_bufs=1, swdge_frac=0.0, dma_ahead=3):
    assert sum(plan) == COLS
    NL = len(plan)
    NACC = 2 * NL if pe_final else NL
    per_core = COLS * P

    nc = bacc.Bacc()
    src_out = nc.declare_dram_parameter("output_pose", [per_core, 72], F32,
                                        isOutput=False)
    src_gt = nc.declare_dram_parameter("gt_pose", [per_core, 72], F32,
                                       isOutput=False)
    if pe_final:
        # [I | -I] fp16 stationaries for the PE d-accumulation
        identpm_in = nc.declare_dram_parameter("identpm", [P, 2 * P],
                                               mybir.dt.float16,
                                               isOutput=False)
    acc_out = nc.declare_dram_parameter("acc_out", [P, NACC], F32,
                                        isOutput=True)

    DT = dve_dtype

    import contextlib
    with tile.TileContext(nc) as tc:
        loop_ctx = tc.For_i(0, loop, 1) if loop else contextlib.nullcontext()
        with (
            loop_ctx,
            tc.tile_pool(name="m32_pool", bufs=m32_bufs) as m32pool,
            tc.tile_pool(name="m16_pool", bufs=m16_bufs) as m16pool,
            tc.tile_pool(name="x_pool", bufs=x_bufs) as xpool,
            tc.tile_pool(name="v_pool", bufs=v_bufs) as vpool,
            tc.tile_pool(name="s_pool", bufs=s_bufs) as spool,
            tc.tile_pool(name="acc_pool", bufs=1) as accpool,
            tc.tile_pool(name="psum_pool", bufs=4, space="PSUM") as ppool,
        ):
            acc = accpool.tile([P, NACC], F32)
            if pe_final:
                identpm = accpool.tile([P, 2 * P], mybir.dt.float16)
                nc.gpsimd.dma_start(out=identpm[:], in_=identpm_in[:])

            lanes = []
            col_base = 0
            for li, S in enumerate(plan):
                lanes.append((li, S, col_base))
                col_base += S

            # ---- DMA issue: stagger K lanes ahead. The HWDGE ring has 16
            # channels and round-robins queued transfers, so issuing ALL
            # lanes up front makes every lane finish together (no early
            # data, no overlap). K in flight => lane l lands ~K transfers
            # after its issue while the ring stays fed. ----
            lane_m32 = {}

            def issue_dma(li):
                S = plan[li]
                cb = sum(plan[:li])
                row0 = cb * P
                pair = []
                for a, (src, ring) in enumerate(
                    ((src_out, nc.sync), (src_gt, nc.scalar))
                ):
                    m32 = m32pool.tile([P, S * 72], F32, tag=f"m32_{a}",
                                       name=f"m32_{a}")
                    if stages != "compute":
                        srcv = src[row0: row0 + P * S, :].rearrange(
                            "(p s) f -> p (s f)", p=P)
                        if swdge_frac > 0.0:
                            # offload the tail of each load to the otherwise
                            # idle gpsimd SWDGE ring (~170 GB/s measured)
                            h = int(S * (1.0 - swdge_frac)) * 72
                            ring.dma_start(out=m32[:, :h], in_=srcv[:, :h])
                            nc.gpsimd.dma_start(out=m32[:, h:],
                                                in_=srcv[:, h:])
                        else:
                            ring.dma_start(out=m32[:], in_=srcv)
                    pair.append(m32)
                lane_m32[li] = pair

            DMA_AHEAD = min(dma_ahead, NL)
            for li in range(DMA_AHEAD):
                issue_dma(li)

            def emit_conv(li, S):
                # ScalarE fp32 -> fp16 transposed conversion.
                # m16 [P, 4tc, 30, S]; tensor a covers tc in {2a, 2a+1}.
                # out APs keep s innermost (unit stride); in APs may have
                # any inner stride (s steps by 72 in the raw layout).
                m32s = lane_m32[li]
                m16 = m16pool.tile([P, 4, 30, S], DT, tag="m16")
                for a in range(2):
                    toff = a * 60 * S
                    nc.scalar.copy(
                        _ap(m16, [[S, 27], [30 * S, 2], [1, S]], toff),
                        _ap(m32s[a], [[1, 27], [36, 2], [72, S]], 0),
                    )
                    nc.scalar.copy(
                        _ap(m16, [[S, 3], [30 * S, 2], [1, S]],
                            toff + 27 * S),
                        _ap(m32s[a], [[3, 3], [36, 2], [72, S]], 29),
                    )
                return m16

            def emit_compute(li, S, m16):
                def mcol(d, k):
                    """m16 column k of depth d: dims (tc4, r3, S)."""
                    return _ap(m16, [[30 * S, 4], [3 * S, 3], [1, S]],
                               (9 * d + k) * S)

                # DVE: cross products x = c0 x c1, depths 0..2
                # x16 [P, 4tc, 3r, 3d, S]
                x16 = xpool.tile([P, 4, 3, 3, S], DT, tag="x")
                tmp = spool.tile([P, 4, 3, S], DT, tag="tmp")
                for r in range(3):
                    r1, r2 = (r + 1) % 3, (r + 2) % 3
                    dims_in = [[30 * S, 4], [9 * S, 3], [1, S]]  # (tc, d, S)
                    xr = _ap(x16, [[9 * S, 4], [S, 3], [1, S]], r * 3 * S)
                    nc.vector.tensor_mul(
                        xr,
                        _ap(m16, dims_in, (3 * r1 + 0) * S),
                        _ap(m16, dims_in, (3 * r2 + 1) * S))
                    nc.vector.tensor_mul(
                        tmp[:],
                        _ap(m16, dims_in, (3 * r2 + 0) * S),
                        _ap(m16, dims_in, (3 * r1 + 1) * S))
                    nc.vector.tensor_sub(xr, xr, tmp[:])
                if stages == "cross":
                    return

                # DVE chain: v <- R_d v + t_d, d = 2, 1, 0
                # v tiles [P, 4tc, 3r, S]
                def vin_t3(j):
                    return _ap(m16, [[30 * S, 4], [0, 3], [1, S]],
                               (27 + j) * S)

                def vin_v(vt, j):
                    return _ap(vt, [[3 * S, 4], [0, 3], [1, S]], j * S)

                def xd(d):
                    return _ap(x16, [[9 * S, 4], [3 * S, 3], [1, S]], d * S)

                p0 = spool.tile([P, 4, 3, S], DT, tag="p0")
                p1 = spool.tile([P, 4, 3, S], DT, tag="p1")
                va = vpool.tile([P, 4, 3, S], DT, tag="va")
                vb = vpool.tile([P, 4, 3, S], DT, tag="vb")

                def step(d, vin, vout):
                    nc.vector.tensor_mul(p0[:], xd(d), vin(0))
                    nc.vector.tensor_mul(p1[:], mcol(d, 0), vin(1))
                    nc.vector.tensor_add(p0[:], p0[:], p1[:])
                    nc.vector.tensor_mul(p1[:], mcol(d, 1), vin(2))
                    nc.vector.tensor_add(p0[:], p0[:], p1[:])
                    nc.vector.tensor_add(vout[:], p0[:], mcol(d, 2))

                step(2, vin_t3, va)
                step(1, lambda j: vin_v(va, j), vb)

                if not pe_final:
                    z = vpool.tile([P, 4, 3, S], DT, tag="z")
                    step(0, lambda j: vin_v(vb, j), z)
                    # loss: acc[:, li] = sum of (z0 - z1)^2
                    # z [P, 4tc, 3r, S]: out half tc in {0,1}, gt {2,3}
                    d16 = spool.tile([P, 2, 3, S], DT, tag="d16")
                    dsq = spool.tile([P, 2, 3, S], F32, tag="dsq")
                    zdims = [[3 * S, 2], [S, 3], [1, S]]
                    nc.vector.tensor_sub(d16[:], _ap(z, zdims, 0),
                                         _ap(z, zdims, 6 * S))
                    nc.scalar.activation(
                        dsq[:], d16[:], mybir.ActivationFunctionType.Square,
                        accum_out=acc[:, li: li + 1])
                else:
                    # step 0 products on DVE; d = z_out - z_gt accumulated
                    # directly in PSUM via [I | -I] stationaries.
                    px = spool.tile([P, 4, 3, S], DT, tag="px")
                    pc0 = spool.tile([P, 4, 3, S], DT, tag="pc0")
                    pc1 = spool.tile([P, 4, 3, S], DT, tag="pc1")
                    nc.vector.tensor_mul(px[:], xd(0), vin_v(vb, 0))
                    nc.vector.tensor_mul(pc0[:], mcol(0, 0), vin_v(vb, 1))
                    nc.vector.tensor_mul(pc1[:], mcol(0, 1), vin_v(vb, 2))
                    for c in range(2):
                        pd = ppool.tile([P, 3, S], F32, tag=f"pd{li % 4}_{c}",
                                        bufs=1, name=f"pd{li}_{c}")
                        first = True
                        for a in range(2):
                            tcix = 2 * a + c
                            stat = identpm[:, a * P:(a + 1) * P]
                            movs = [
                                _ap(px, [[S, 3], [1, S]], tcix * 3 * S),
                                _ap(pc0, [[S, 3], [1, S]], tcix * 3 * S),
                                _ap(pc1, [[S, 3], [1, S]], tcix * 3 * S),
                                _ap(m16, [[3 * S, 3], [1, S]],
                                    tcix * 30 * S + 2 * S),
                            ]
                            for mi, mov in enumerate(movs):
                                nc.tensor.matmul(
                                    pd[:], stat, mov, start=first,
                                    stop=(a == 1 and mi == 3))
                                first = False
                        pending_sq.append((pd, 2 * li + c))

            # ---- Phase B/C: conv skewed one lane ahead of compute;
            # squares deferred to the end so the in-order Act queue never
            # makes a later conv wait on an earlier lane's DVE+PE chain ----
            pending_sq = []
            if stages == "dma":
                for li in range(DMA_AHEAD, NL):
                    issue_dma(li)
            else:
                lane_m16 = {0: emit_conv(0, plan[0])}
                Smax = max(plan)
                dsq = spool.tile([P, 3, Smax], F32, tag="dsq")

                def drain_squares():
                    for pd, col in pending_sq:
                        nc.scalar.activation(
                            dsq[:, :, 0:pd.shape[-1]], pd[:],
                            mybir.ActivationFunctionType.Square,
                            accum_out=acc[:, col: col + 1])
                    pending_sq.clear()

                for li, S, cb in lanes:
                    if li + DMA_AHEAD < NL:
                        issue_dma(li + DMA_AHEAD)
                    if li + 1 < NL:
                        lane_m16[li + 1] = emit_conv(li + 1, plan[li + 1])
                    if stages != "conv":
                        emit_compute(li, S, lane_m16[li])
                    if len(pending_sq) >= 8:
                        drain_squares()
                drain_squares()

            if stages == "full":
                nc.gpsimd.dma_start(out=acc_out[:], in_=acc[:])
    nc.finalize()
    return nc


_NC_CACHE = {}


def _get_nc():
    if 'nc' not in _NC_CACHE:
        _NC_CACHE['nc'] = build_nc()
    return _NC_CACHE['nc']


def make_in_maps(output_pose, gt_pose, pe_final=True):
    op = np.ascontiguousarray(output_pose, dtype=np.float32)
    gt = np.ascontiguousarray(gt_pose, dtype=np.float32)
    maps = [
        {
            "output_pose": op[c * PER_CORE: (c + 1) * PER_CORE],
            "gt_pose": gt[c * PER_CORE: (c + 1) * PER_CORE],
        }
        for c in range(N_CORES)
    ]
    if pe_final:
        ident = np.eye(P, dtype=np.float16)
        identpm = np.concatenate([ident, -ident], axis=1)
        for m in maps:
            m["identpm"] = identpm
    return maps


def kernel(output_pose, gt_pose, gt_prev_pose=None, **_ignored):
    from concourse.bass_utils import run_bass_kernel_spmd
    nc = _get_nc()
    in_maps = make_in_maps(output_pose, gt_pose)
    res = run_bass_kernel_spmd(nc, in_maps, list(range(N_CORES)))
    total = 0.0
    for r in res.results:
        total += float(np.sum(r["acc_out"].astype(np.float64)))
    loss = np.float32(total / (B * 6))
    return (loss, loss)
